# revision 17
# baseline (speedup 1.0000x reference)
"""Trainium2 Bass kernel for nn_Decoder (LAS-style attention decoder).

Sharding: data-parallel over batch N=32 across 8 cores (4 examples/core).
Per core, three phases in one SPMD program:
  A) batched precompute: k/v projections, char-embedding gather (teacher
     forcing makes all decoder inputs known), attention-1 for all steps,
     and the input-side LSTM1 pre-gates, batched over all 200 steps.
  B) the only truly sequential part: the LSTM1+LSTM2 recurrence. The
     scan carry never touches attention-2, so only the two cells run
     step-by-step. Gates are computed W-moving (fp32r) into 4 psum
     quarter-blocks at partition bases {0,32,64,96} so the elementwise
     chain runs on [128, small] shapes.
  C) batched postprocess: q2, attention-2, fc+hardtanh, and the tied
     embedding logit GEMM.

Gate reordering (host): torch gate order [i,f,g,o] is regrouped into 4
quarter blocks each laid out [i_q f_q o_q g_q]; g rows are pre-scaled by
2 so one sigmoid pass serves all gates (tanh(g) = 2*sigmoid(2g)-1).
"""
import json
import sys

sys.path.insert(0, "/opt/trn_rl_repo")
sys.path.insert(0, "/root/.axon_site")

import numpy as np

N, T, L = 32, 512, 200
VOCAB, EMB, KEY, DEC_H, ENC_OUT = 10000, 256, 256, 512, 1024
NCORES = 8
EXPC = N // NCORES
LP = 256                    # L padded for fp32r-friendly free dims
LB = [(0, 128), (128, 72)]  # L tiles (offset, size)
NEG = -1e9


def _fix_sync_waits(bir_json: bytes, max_waits: int = 1) -> bytes:
    """This container's walrus rejects >1 sync wait per instruction;
    split extras into preceding same-engine NoOps."""
    m = json.loads(bir_json)
    cnt = [0]
    for fn in m["functions"]:
        for blk in fn["blocks"]:
            out = []
            for inst in blk.get("instructions", []):
                si = inst.get("sync_info") or {}
                waits = si.get("on_wait") or []
                if len(waits) > max_waits:
                    keep = waits[-max_waits:]
                    spill = waits[:-max_waits]
                    for i in range(0, len(spill), max_waits):
                        cnt[0] += 1
                        out.append({
                            "debug": inst.get("debug", 0),
                            "engine": inst["engine"],
                            "ins": [], "outs": [],
                            "name": f"wsp-{cnt[0]}-{inst['name']}",
                            "opcode": "NoOp",
                            "sync_info": {"on_update": [],
                                          "on_wait": spill[i:i + max_waits]},
                        })
                    si["on_wait"] = keep
                out.append(inst)
            blk["instructions"] = out
    return json.dumps(m).encode()


def _gate_perm(H, nb):
    """Regroup [i,f,g,o] (each H wide) into nb blocks laid out
    [i_b f_b o_b g_b], block width H//nb."""
    q = H // nb
    perm = []
    for j in range(nb):
        perm += list(range(j * q, (j + 1) * q))                  # i_b
        perm += list(range(H + j * q, H + (j + 1) * q))          # f_b
        perm += list(range(3 * H + j * q, 3 * H + (j + 1) * q))  # o_b
        perm += list(range(2 * H + j * q, 2 * H + (j + 1) * q))  # g_b
    return np.array(perm)


def _softmax(nc, pool, eps, att, ln):
    """Softmax along free dim T (mask already added into eps by caller);
    eps [128, T] psum (rows :ln valid), att out sbuf [128, T]."""
    from concourse import mybir
    AX = mybir.AxisListType.X
    EXP = mybir.ActivationFunctionType.Exp
    ALU = mybir.AluOpType
    F32 = mybir.dt.float32
    red = pool.tile([128, 1], F32, tag="sm_red")
    nc.vector.reduce_max(red[:ln, :], eps[:ln, :], axis=AX)
    nc.vector.tensor_scalar(eps[:ln, :], eps[:ln, :], red[:ln, :], None,
                            op0=ALU.subtract)
    nc.scalar.activation(att[:ln, :], eps[:ln, :], EXP)
    s = pool.tile([128, 1], F32, tag="sm_s")
    nc.vector.reduce_sum(s[:ln, :], att[:ln, :], axis=AX)
    rs = pool.tile([128, 1], F32, tag="sm_rs")
    nc.vector.reciprocal(rs[:ln, :], s[:ln, :])
    nc.vector.tensor_scalar(att[:ln, :], att[:ln, :], rs[:ln, :], None,
                            op0=ALU.mult)


def _build(nsteps):
    import concourse.bass as bass
    import concourse.tile as tile
    from concourse import mybir
    from contextlib import ExitStack

    F32 = mybir.dt.float32
    F32R = mybir.dt.float32r
    I32 = mybir.dt.int32
    SIG = mybir.ActivationFunctionType.Sigmoid
    TANH = mybir.ActivationFunctionType.Tanh
    ALU = mybir.AluOpType

    nc = bass.Bass()
    dt_ = lambda name, shp, dt=F32: nc.dram_tensor(name, shp, dt, kind="ExternalInput")
    enc = dt_("enc", [EXPC, T, ENC_OUT])
    toks = dt_("toks", [1024, 1], I32)
    maskadd = dt_("maskadd", [1, EXPC * T])
    E_h = dt_("E_h", [VOCAB, EMB])
    ET_h = dt_("ET_h", [128, 2, VOCAB])
    Wk_h = dt_("Wk_h", [128, 8, KEY])
    Wv_h = dt_("Wv_h", [128, 8, KEY])
    Wq_h = dt_("Wq_h", [128, 2, KEY])
    Wih1T_h = dt_("Wih1T_h", [128, 4, 4 * DEC_H])
    Whh1T_h = dt_("Whh1T_h", [128, 4, 4 * DEC_H])
    b1_h = dt_("b1_h", [1, 4 * DEC_H])
    Wih2T_h = dt_("Wih2T_h", [128, 4, 4 * KEY])
    Whh2T_h = dt_("Whh2T_h", [128, 2, 4 * KEY])
    b2_h = dt_("b2_h", [1, 4 * KEY])
    Wfc_h = dt_("Wfc_h", [128, 4, EMB])
    bq_h = dt_("bq_h", [128, 2, 1])
    bfc_h = dt_("bfc_h", [128, 2, 1])
    bchar_h = dt_("bchar_h", [1, VOCAB])
    ident_h = dt_("ident_h", [128, 128])
    identb_h = dt_("identb_h", [128, EXPC])

    preds = nc.dram_tensor("preds", [EXPC, L, VOCAB], F32, kind="ExternalOutput")
    attns = nc.dram_tensor("attns", [L, T], F32, kind="ExternalOutput")

    with tile.TileContext(nc) as tc, ExitStack() as ctx:
        res = ctx.enter_context(tc.tile_pool(name="res", bufs=1))
        dram = ctx.enter_context(tc.tile_pool(name="dram", bufs=1, space="DRAM"))

        ident = res.tile([128, 128], F32)
        nc.gpsimd.dma_start(ident[:], ident_h[:])
        identR = res.tile([EXPC, EXPC], F32R)
        nc.gpsimd.dma_start(identR[:], ident_h[:EXPC, :EXPC])
        identB = res.tile([128, EXPC], F32)
        nc.gpsimd.dma_start(identB[:], identb_h[:])
        KT = res.tile([128, 2, EXPC, T], F32R)     # k^T: [emb_kt, ex, T]
        V = res.tile([128, 4, EXPC, KEY], F32R)    # v: [T-tile, ex, emb]
        H2T = res.tile([128, 2, EXPC, LP], F32R)   # h2^T for all steps
        nc.vector.memset(H2T[:].bitcast(F32), 0.0)
        mask_t = res.tile([1, EXPC * T], F32R)
        nc.gpsimd.dma_start(mask_t[:], maskadd[:])
        onesR = res.tile([1, 128], F32R)
        nc.vector.memset(onesR[:].bitcast(F32), 1.0)
        Wq = res.tile([128, 2, KEY], F32R)
        nc.gpsimd.dma_start(Wq[:], Wq_h[:])
        bq = res.tile([128, 2, 1], F32)
        nc.gpsimd.dma_start(bq[:], bq_h[:])

        PRE1 = dram.tile([EXPC, L, 4 * DEC_H], F32)

        # =========== PHASE A ===========
        with tc.tile_pool(name="pa", bufs=1) as pa, \
             tc.tile_pool(name="pa1", bufs=1) as pa1, \
             tc.tile_pool(name="pa2", bufs=2) as pa2, \
             tc.tile_pool(name="paps", bufs=4, space="PSUM") as paps:
            Wk = pa.tile([128, 8, KEY], F32R)
            nc.gpsimd.dma_start(Wk[:], Wk_h[:])
            Wv = pa.tile([128, 8, KEY], F32R)
            nc.gpsimd.dma_start(Wv[:], Wv_h[:])

            for ex in range(EXPC):
                encF = pa1.tile([128, 8, T], F32, tag="encF")
                for a in range(8):
                    nc.sync.dma_start(
                        encF[:, a, :],
                        enc[ex, :, a * 128:(a + 1) * 128].rearrange("t p -> p t"))
                encT = pa1.tile([128, 8, T], F32R, tag="encT")
                nc.vector.tensor_copy(encT[:], encF[:])
                for mt in range(2):
                    ps = paps.tile([128, T], F32, space="PSUM", tag="ps")
                    for kt in range(8):
                        nc.tensor.matmul(
                            ps[:], (Wk[:, kt, mt * 128:(mt + 1) * 128]),
                            (encT[:, kt, :]),
                            start=(kt == 0), stop=(kt == 7))
                    nc.vector.tensor_copy(KT[:, mt, ex, :], ps[:])
                for tt in range(4):
                    ps = paps.tile([128, KEY], F32, space="PSUM", tag="ps")
                    for kt in range(8):
                        nc.tensor.matmul(
                            ps[:], (encT[:, kt, tt * 128:(tt + 1) * 128]),
                            (Wv[:, kt, :]),
                            start=(kt == 0), stop=(kt == 7))
                    nc.vector.tensor_copy(V[:, tt, ex, :], ps[:])

            # char embedding gather + transpose into INPT tiles 0-1
            INPT = pa.tile([128, 4, EXPC, LP], F32R)
            nc.vector.memset(INPT[:].bitcast(F32), 0.0)
            tok_t = pa.tile([128, 8, 1], I32)
            nc.gpsimd.dma_start(
                tok_t[:], toks.rearrange("(a p) b -> p a b", p=128))
            ce = pa.tile([128, 8, EMB], F32)
            for g in range(8):
                nc.gpsimd.indirect_dma_start(
                    out=ce[:, g, :], out_offset=None, in_=E_h[:],
                    in_offset=bass.IndirectOffsetOnAxis(ap=tok_t[:, g, :], axis=0))
            for ex in range(EXPC):
                for lt, (lo, ln) in enumerate(LB):
                    col = ex * 2 + lt
                    for et in range(2):
                        ps = paps.tile([128, 128], F32, space="PSUM", tag="ps")
                        nc.tensor.transpose(
                            ps[:, :ln], ce[:ln, col, et * 128:(et + 1) * 128],
                            ident[:ln, :ln])
                        nc.vector.tensor_copy(INPT[:, et, ex, lo:lo + ln],
                                              ps[:, :ln])

            # Q1^T (+bq)
            Q1T = pa.tile([128, 2, EXPC, LP], F32R)
            for ex in range(EXPC):
                for mt in range(2):
                    ps = paps.tile([128, LP], F32, space="PSUM", tag="ps")
                    for kt in range(2):
                        nc.tensor.matmul(
                            ps[:], (Wq[:, kt, mt * 128:(mt + 1) * 128]),
                            (INPT[:, kt, ex, :]),
                            start=(kt == 0), stop=(kt == 1))
                    nc.vector.tensor_scalar(
                        Q1T[:, mt, ex, :], ps[:], bq[:, mt, :], None, op0=ALU.add)

            # attention 1 (+ ctx1^T per example)
            for ex in range(EXPC):
                AT1T = pa2.tile([128, 4, LP], F32R, tag="at1")
                nc.vector.memset(AT1T[:].bitcast(F32), 0.0)
                for (lo, ln) in LB:
                    eps = paps.tile([128, T], F32, space="PSUM", tag="ps")
                    for kt in range(2):
                        nc.tensor.matmul(
                            eps[:ln, :], (Q1T[:, kt, ex, lo:lo + ln]),
                            (KT[:, kt, ex, :]),
                            start=(kt == 0), stop=False)
                    nc.tensor.matmul(
                        eps[:ln, :], (onesR[:, :ln]),
                        (mask_t[:, ex * T:(ex + 1) * T]),
                        start=False, stop=True)
                    att = pa2.tile([128, T], F32, tag="att")
                    _softmax(nc, pa2, eps, att, ln)
                    for tt in range(4):
                        tps = paps.tile([128, 128], F32, space="PSUM", tag="ps")
                        nc.tensor.transpose(
                            tps[:, :ln], att[:ln, tt * 128:(tt + 1) * 128],
                            ident[:ln, :ln])
                        nc.vector.tensor_copy(
                            AT1T[:, tt, lo:lo + ln], tps[:, :ln])
                # ctx1^T -> INPT tiles 2-3
                for mt in range(2):
                    ps = paps.tile([128, LP], F32, space="PSUM", tag="ps")
                    for kt in range(4):
                        nc.tensor.matmul(
                            ps[:], (V[:, kt, ex, mt * 128:(mt + 1) * 128]),
                            (AT1T[:, kt, :]),
                            start=(kt == 0), stop=(kt == 3))
                    nc.vector.tensor_copy(INPT[:, 2 + mt, ex, :], ps[:])

            # LSTM1 pre-gates -> PRE1 (HBM)
            Wih1T = pa.tile([128, 4, 4 * DEC_H], F32R)
            nc.gpsimd.dma_start(Wih1T[:], Wih1T_h[:])
            b1 = pa.tile([1, 4 * DEC_H], F32R)
            nc.gpsimd.dma_start(b1[:], b1_h[:])
            for ex in range(EXPC):
                for (lo, ln) in LB:
                    for ch in range(4):
                        ps = paps.tile([128, 512], F32, space="PSUM", tag="ps")
                        for kt in range(4):
                            nc.tensor.matmul(
                                ps[:ln, :], (INPT[:, kt, ex, lo:lo + ln]),
                                (Wih1T[:, kt, ch * 512:(ch + 1) * 512]),
                                start=(kt == 0), stop=False)
                        nc.tensor.matmul(
                            ps[:ln, :], (onesR[:, :ln]),
                            (b1[:, ch * 512:(ch + 1) * 512]),
                            start=False, stop=True)
                        st = pa2.tile([128, 512], F32, tag="prest")
                        nc.vector.tensor_copy(st[:ln, :], ps[:ln, :])
                        nc.sync.dma_start(
                            PRE1[ex, lo:lo + ln, ch * 512:(ch + 1) * 512],
                            st[:ln, :])

        # =========== PHASE B: recurrence ===========
        with tc.tile_pool(name="pb", bufs=1) as pb, \
             tc.tile_pool(name="pb3", bufs=2) as pb3, \
             tc.tile_pool(name="pbps", bufs=1, space="PSUM") as pbps, \
             tc.tile_pool(name="pbpt", bufs=2, space="PSUM") as pbpt:
            Whh1T = pb.tile([128, 4, 4 * DEC_H], F32R)
            nc.gpsimd.dma_start(Whh1T[:], Whh1T_h[:])
            Wih2T = pb.tile([128, 4, 4 * KEY], F32R)
            nc.gpsimd.dma_start(Wih2T[:], Wih2T_h[:])
            Whh2T = pb.tile([128, 2, 4 * KEY], F32R)
            nc.gpsimd.dma_start(Whh2T[:], Whh2T_h[:])
            b2 = pb.tile([1, 4 * KEY], F32R)
            nc.gpsimd.dma_start(b2[:], b2_h[:])
            ones1 = pb.tile([1, 4], F32R)
            nc.vector.memset(ones1[:].bitcast(F32), 1.0)

            h1T = pb.tile([128, 4, EXPC], F32R)
            nc.vector.memset(h1T[:].bitcast(F32), 0.0)
            c1 = pb.tile([EXPC, 512], F32)
            nc.vector.memset(c1[:], 0.0)
            c2 = pb.tile([EXPC, 256], F32)
            nc.vector.memset(c2[:], 0.0)

            for l in range(nsteps):
                pre_l = pb3.tile([EXPC, 4 * DEC_H], F32R, tag="prel")
                nc.gpsimd.dma_start(pre_l[:], PRE1[:, l, :])

                # LSTM1 gates: [4, 2048] psum (layout [i,f,o,g])
                g1 = pbps.tile([EXPC, 2048], F32, space="PSUM", tag="g1")
                for ch in range(4):
                    o = g1[:, ch * 512:(ch + 1) * 512]
                    nc.tensor.matmul(
                        o, identR[:], (pre_l[:, ch * 512:(ch + 1) * 512]),
                        start=True, stop=False)
                    for kt in range(4):
                        nc.tensor.matmul(
                            o, (h1T[:, kt, :]),
                            (Whh1T[:, kt, ch * 512:(ch + 1) * 512]),
                            start=False, stop=(kt == 3))
                s1 = pb3.tile([EXPC, 2048], F32, tag="s1")
                nc.scalar.activation(s1[:], g1[:], SIG)
                t1 = pb3.tile([EXPC, 512], F32, tag="t1")
                nc.vector.tensor_scalar(t1[:], s1[:, 1536:2048], 2.0, -1.0,
                                        op0=ALU.mult, op1=ALU.add)
                nc.vector.tensor_mul(t1[:], t1[:], s1[:, 0:512])
                m1 = pb3.tile([EXPC, 512], F32, tag="m1")
                nc.vector.tensor_mul(m1[:], s1[:, 512:1024], c1[:])
                nc.vector.tensor_add(c1[:], m1[:], t1[:])
                th1 = pb3.tile([EXPC, 512], F32, tag="th1")
                nc.scalar.activation(th1[:], c1[:], TANH)
                h1q = pb3.tile([EXPC, 512], F32, tag="h1q")
                nc.vector.tensor_mul(h1q[:], th1[:], s1[:, 1024:1536])
                for kt in range(4):
                    tps = pbpt.tile([128, EXPC], F32, space="PSUM", tag="tp")
                    nc.tensor.transpose(
                        tps[:], h1q[:, kt * 128:(kt + 1) * 128],
                        ident[:EXPC, :EXPC])
                    nc.vector.tensor_copy(h1T[:, kt, :], tps[:])

                # LSTM2 gates: [4, 1024] psum
                g2 = pbps.tile([EXPC, 1024], F32, space="PSUM", tag="g2")
                h2prev = H2T[:, :, :, l - 1] if l > 0 else H2T[:, :, :, LP - 1]
                for ch in range(2):
                    o = g2[:, ch * 512:(ch + 1) * 512]
                    nc.tensor.matmul(
                        o, (ones1[:]), (b2[:, ch * 512:(ch + 1) * 512]),
                        start=True, stop=False)
                    for kt in range(4):
                        nc.tensor.matmul(
                            o, (h1T[:, kt, :]),
                            (Wih2T[:, kt, ch * 512:(ch + 1) * 512]),
                            start=False, stop=False)
                    for kt in range(2):
                        nc.tensor.matmul(
                            o, (h2prev[:, kt, :]),
                            (Whh2T[:, kt, ch * 512:(ch + 1) * 512]),
                            start=False, stop=(kt == 1))
                s2 = pb3.tile([EXPC, 1024], F32, tag="s2")
                nc.scalar.activation(s2[:], g2[:], SIG)
                t2 = pb3.tile([EXPC, 256], F32, tag="t2")
                nc.vector.tensor_scalar(t2[:], s2[:, 768:1024], 2.0, -1.0,
                                        op0=ALU.mult, op1=ALU.add)
                nc.vector.tensor_mul(t2[:], t2[:], s2[:, 0:256])
                m2 = pb3.tile([EXPC, 256], F32, tag="m2")
                nc.vector.tensor_mul(m2[:], s2[:, 256:512], c2[:])
                nc.vector.tensor_add(c2[:], m2[:], t2[:])
                th2 = pb3.tile([EXPC, 256], F32, tag="th2")
                nc.scalar.activation(th2[:], c2[:], TANH)
                h2q = pb3.tile([EXPC, 256], F32, tag="h2q")
                nc.vector.tensor_mul(h2q[:], th2[:], s2[:, 512:768])
                for half in range(2):
                    tps = pbpt.tile([128, EXPC], F32, space="PSUM", tag="tp")
                    nc.tensor.transpose(
                        tps[:], h2q[:, half * 128:(half + 1) * 128],
                        ident[:EXPC, :EXPC])
                    nc.vector.tensor_copy(H2T[:, half, :, l], tps[:])

        # =========== PHASE C ===========
        with tc.tile_pool(name="pc", bufs=1) as pcp, \
             tc.tile_pool(name="pc2", bufs=2) as pc2, \
             tc.tile_pool(name="pc4", bufs=4) as pc4, \
             tc.tile_pool(name="pcps", bufs=4, space="PSUM") as pcps:
            Q2T = pcp.tile([128, 2, EXPC, LP], F32R)
            for ex in range(EXPC):
                for mt in range(2):
                    ps = pcps.tile([128, LP], F32, space="PSUM", tag="ps")
                    for kt in range(2):
                        nc.tensor.matmul(
                            ps[:], (Wq[:, kt, mt * 128:(mt + 1) * 128]),
                            (H2T[:, kt, ex, :]),
                            start=(kt == 0), stop=(kt == 1))
                    nc.vector.tensor_scalar(
                        Q2T[:, mt, ex, :], ps[:], bq[:, mt, :], None, op0=ALU.add)

            CT2T = pcp.tile([128, 2, EXPC, LP], F32R)
            for ex in range(EXPC):
                AT2T = pc2.tile([128, 4, LP], F32R, tag="at2")
                nc.vector.memset(AT2T[:].bitcast(F32), 0.0)
                for (lo, ln) in LB:
                    eps = pcps.tile([128, T], F32, space="PSUM", tag="ps")
                    for kt in range(2):
                        nc.tensor.matmul(
                            eps[:ln, :], (Q2T[:, kt, ex, lo:lo + ln]),
                            (KT[:, kt, ex, :]),
                            start=(kt == 0), stop=False)
                    nc.tensor.matmul(
                        eps[:ln, :], (onesR[:, :ln]),
                        (mask_t[:, ex * T:(ex + 1) * T]),
                        start=False, stop=True)
                    att = pc2.tile([128, T], F32, tag="att2")
                    _softmax(nc, pc2, eps, att, ln)
                    if ex == 0:
                        nc.sync.dma_start(attns[lo:lo + ln, :], att[:ln, :])
                    for tt in range(4):
                        tps = pcps.tile([128, 128], F32, space="PSUM", tag="ps")
                        nc.tensor.transpose(
                            tps[:, :ln], att[:ln, tt * 128:(tt + 1) * 128],
                            ident[:ln, :ln])
                        nc.vector.tensor_copy(
                            AT2T[:, tt, lo:lo + ln], tps[:, :ln])
                for mt in range(2):
                    ps = pcps.tile([128, LP], F32, space="PSUM", tag="ps")
                    for kt in range(4):
                        nc.tensor.matmul(
                            ps[:], (V[:, kt, ex, mt * 128:(mt + 1) * 128]),
                            (AT2T[:, kt, :]),
                            start=(kt == 0), stop=(kt == 3))
                    nc.vector.tensor_copy(CT2T[:, mt, ex, :], ps[:])

            # fco^T = clip(Wfc.T @ [h2; ctx2] + bfc, -1, 1)
            Wfc = pcp.tile([128, 4, EMB], F32R)
            nc.gpsimd.dma_start(Wfc[:], Wfc_h[:])
            bfc = pcp.tile([128, 2, 1], F32)
            nc.gpsimd.dma_start(bfc[:], bfc_h[:])
            FCOT = pcp.tile([128, 2, EXPC, LP], F32R)
            for ex in range(EXPC):
                for mt in range(2):
                    ps = pcps.tile([128, LP], F32, space="PSUM", tag="ps")
                    for kt in range(2):
                        nc.tensor.matmul(
                            ps[:], (Wfc[:, kt, mt * 128:(mt + 1) * 128]),
                            (H2T[:, kt, ex, :]),
                            start=(kt == 0), stop=False)
                    for kt in range(2):
                        nc.tensor.matmul(
                            ps[:], (Wfc[:, 2 + kt, mt * 128:(mt + 1) * 128]),
                            (CT2T[:, kt, ex, :]),
                            start=False, stop=(kt == 1))
                    ftmp = pc2.tile([128, LP], F32, tag="ftmp")
                    nc.vector.tensor_scalar(ftmp[:], ps[:], bfc[:, mt, :], 1.0,
                                            op0=ALU.add, op1=ALU.min)
                    nc.vector.tensor_scalar_max(FCOT[:, mt, ex, :], ftmp[:], -1.0)

            # logits: vc-outer, ET/bchar chunks streamed from HBM
            onesL = pcp.tile([1, 128], F32R)
            nc.vector.memset(onesL[:].bitcast(F32), 1.0)
            for vc in range(20):
                vn = 512 if vc < 19 else VOCAB - 19 * 512
                ET = pc2.tile([128, 2, 512], F32R, tag="etc")
                nc.gpsimd.dma_start(ET[:, :, :vn],
                                    ET_h[:, :, vc * 512:vc * 512 + vn])
                bchar = pc2.tile([1, 512], F32R, tag="bch")
                nc.gpsimd.dma_start(bchar[:, :vn],
                                    bchar_h[:, vc * 512:vc * 512 + vn])
                for ex in range(EXPC):
                    for (lo, ln) in LB:
                        ps = pcps.tile([128, 512], F32, space="PSUM", tag="ps")
                        nc.tensor.matmul(
                            ps[:ln, :vn], (onesL[:, :ln]),
                            (bchar[:, :vn]),
                            start=True, stop=False)
                        for kt in range(2):
                            nc.tensor.matmul(
                                ps[:ln, :vn], (FCOT[:, kt, ex, lo:lo + ln]),
                                (ET[:, kt, :vn]),
                                start=False, stop=(kt == 1))
                        st = pc4.tile([128, 512], F32, tag="lgst")
                        if ex % 2 == 0:
                            nc.scalar.copy(st[:ln, :vn], ps[:ln, :vn])
                        else:
                            nc.vector.tensor_copy(st[:ln, :vn], ps[:ln, :vn])
                        nc.sync.dma_start(
                            preds[ex, lo:lo + ln, vc * 512:vc * 512 + vn],
                            st[:ln, :vn])

    orig = nc.to_json_bytes
    nc.to_json_bytes = lambda: _fix_sync_waits(orig())
    return nc


def _prep(inputs):
    f = np.float32
    enc = np.asarray(inputs["encoder_out"], f)
    text = np.asarray(inputs["text"]).astype(np.int32)
    lens = np.asarray(inputs["lens"]).astype(np.int64)
    E = np.asarray(inputs["E"], f)
    perm1 = _gate_perm(DEC_H, 1)
    perm2 = _gate_perm(KEY, 1)

    def t_kt(w):  # [in, out] -> [128, in//128, out]
        return np.ascontiguousarray(
            w.reshape(-1, 128, w.shape[1]).transpose(1, 0, 2))

    W1 = np.asarray(inputs["W_ih1"], f)[perm1]
    Wh1 = np.asarray(inputs["W_hh1"], f)[perm1]
    b1 = (np.asarray(inputs["b_ih1"], f) + np.asarray(inputs["b_hh1"], f))[perm1]
    W2 = np.asarray(inputs["W_ih2"], f)[perm2]
    Wh2 = np.asarray(inputs["W_hh2"], f)[perm2]
    b2 = (np.asarray(inputs["b_ih2"], f) + np.asarray(inputs["b_hh2"], f))[perm2]
    # pre-scale g rows by 2 (sigmoid-only trick); layout [i,f,o,g]
    W1[1536:2048] *= 2.0; Wh1[1536:2048] *= 2.0; b1[1536:2048] *= 2.0
    W2[768:1024] *= 2.0; Wh2[768:1024] *= 2.0; b2[768:1024] *= 2.0

    com = {
        "E_h": E,
        "ET_h": np.ascontiguousarray(E.reshape(VOCAB, 2, 128).transpose(2, 1, 0)),
        "Wk_h": t_kt(np.asarray(inputs["Wk"], f)),
        "Wv_h": t_kt(np.asarray(inputs["Wv"], f)),
        "Wq_h": t_kt(np.asarray(inputs["Wq"], f)),
        "Wih1T_h": t_kt(np.ascontiguousarray(W1.T)),
        "Whh1T_h": t_kt(np.ascontiguousarray(Wh1.T)),
        "b1_h": b1.reshape(1, -1),
        "Wih2T_h": t_kt(np.ascontiguousarray(W2.T)),
        "Whh2T_h": t_kt(np.ascontiguousarray(Wh2.T)),
        "b2_h": b2.reshape(1, -1),
        "Wfc_h": t_kt(np.asarray(inputs["Wfc"], f)),
        "bq_h": np.ascontiguousarray(
            np.asarray(inputs["bq"], f).reshape(2, 128).T).reshape(128, 2, 1),
        "bfc_h": np.ascontiguousarray(
            np.asarray(inputs["bfc"], f).reshape(2, 128).T).reshape(128, 2, 1),
        "bchar_h": np.asarray(inputs["b_char"], f).reshape(1, -1),
        "ident_h": np.eye(128, dtype=f),
        "identb_h": np.tile(np.eye(EXPC, dtype=f), (32, 1)),
    }
    tok = np.zeros((N, L), np.int32)
    tok[:, 1:] = text[:, :L - 1]
    mrow = np.arange(T)[None, :] >= lens[:, None]
    madd = np.where(mrow, np.float32(NEG), np.float32(0.0)).astype(f)

    in_maps = []
    for c in range(NCORES):
        sl = slice(c * EXPC, (c + 1) * EXPC)
        tk = np.zeros((1024, 1), np.int32)
        tksub = tok[sl]
        for ex in range(EXPC):
            for lt, (lo, ln) in enumerate(LB):
                col = ex * 2 + lt
                tk[col * 128:col * 128 + ln, 0] = tksub[ex, lo:lo + ln]
        m = dict(com)
        m["enc"] = np.ascontiguousarray(enc[sl])
        m["toks"] = tk
        m["maskadd"] = np.ascontiguousarray(madd[sl].reshape(1, -1))
        in_maps.append(m)
    return in_maps


def kernel(**inputs):
    from concourse.bass_utils import run_bass_kernel_spmd
    nc = _build(L)
    in_maps = _prep(inputs)
    res = run_bass_kernel_spmd(nc, in_maps, core_ids=list(range(NCORES)))
    preds = np.concatenate([r["preds"] for r in res.results], axis=0)
    attns = res.results[0]["attns"]
    return preds, attns


# revision 18
# speedup vs baseline: 1.0050x; 1.0050x over previous
"""Trainium2 Bass kernel for nn_Decoder (LAS-style attention decoder).

Sharding: data-parallel over batch N=32 across 8 cores (4 examples/core).
Per core, three phases in one SPMD program:
  A) batched precompute: k/v projections, char-embedding gather (teacher
     forcing makes all decoder inputs known), attention-1 for all steps,
     and the input-side LSTM1 pre-gates, batched over all 200 steps.
  B) the only truly sequential part: the LSTM1+LSTM2 recurrence. The
     scan carry never touches attention-2, so only the two cells run
     step-by-step. Gates are computed W-moving (fp32r) into 4 psum
     quarter-blocks at partition bases {0,32,64,96} so the elementwise
     chain runs on [128, small] shapes.
  C) batched postprocess: q2, attention-2, fc+hardtanh, and the tied
     embedding logit GEMM.

Gate reordering (host): torch gate order [i,f,g,o] is regrouped into 4
quarter blocks each laid out [i_q f_q o_q g_q]; g rows are pre-scaled by
2 so one sigmoid pass serves all gates (tanh(g) = 2*sigmoid(2g)-1).
"""
import json
import sys

sys.path.insert(0, "/opt/trn_rl_repo")
sys.path.insert(0, "/root/.axon_site")

import numpy as np

N, T, L = 32, 512, 200
VOCAB, EMB, KEY, DEC_H, ENC_OUT = 10000, 256, 256, 512, 1024
NCORES = 8
EXPC = N // NCORES
LP = 256                    # L padded for fp32r-friendly free dims
LB = [(0, 128), (128, 72)]  # L tiles (offset, size)
NEG = -1e9


def _fix_sync_waits(bir_json: bytes, max_waits: int = 1) -> bytes:
    """This container's walrus rejects >1 sync wait per instruction;
    split extras into preceding same-engine NoOps."""
    m = json.loads(bir_json)
    cnt = [0]
    for fn in m["functions"]:
        for blk in fn["blocks"]:
            out = []
            for inst in blk.get("instructions", []):
                si = inst.get("sync_info") or {}
                waits = si.get("on_wait") or []
                if len(waits) > max_waits:
                    keep = waits[-max_waits:]
                    spill = waits[:-max_waits]
                    for i in range(0, len(spill), max_waits):
                        cnt[0] += 1
                        out.append({
                            "debug": inst.get("debug", 0),
                            "engine": inst["engine"],
                            "ins": [], "outs": [],
                            "name": f"wsp-{cnt[0]}-{inst['name']}",
                            "opcode": "NoOp",
                            "sync_info": {"on_update": [],
                                          "on_wait": spill[i:i + max_waits]},
                        })
                    si["on_wait"] = keep
                out.append(inst)
            blk["instructions"] = out
    return json.dumps(m).encode()


def _gate_perm(H, nb):
    """Regroup [i,f,g,o] (each H wide) into nb blocks laid out
    [i_b f_b o_b g_b], block width H//nb."""
    q = H // nb
    perm = []
    for j in range(nb):
        perm += list(range(j * q, (j + 1) * q))                  # i_b
        perm += list(range(H + j * q, H + (j + 1) * q))          # f_b
        perm += list(range(3 * H + j * q, 3 * H + (j + 1) * q))  # o_b
        perm += list(range(2 * H + j * q, 2 * H + (j + 1) * q))  # g_b
    return np.array(perm)


def _softmax(nc, pool, eps, att, ln):
    """Softmax along free dim T (mask already added into eps by caller);
    eps [128, T] psum (rows :ln valid), att out sbuf [128, T]."""
    from concourse import mybir
    AX = mybir.AxisListType.X
    EXP = mybir.ActivationFunctionType.Exp
    ALU = mybir.AluOpType
    F32 = mybir.dt.float32
    red = pool.tile([128, 1], F32, tag="sm_red")
    nc.vector.reduce_max(red[:ln, :], eps[:ln, :], axis=AX)
    nc.vector.tensor_scalar(eps[:ln, :], eps[:ln, :], red[:ln, :], None,
                            op0=ALU.subtract)
    nc.scalar.activation(att[:ln, :], eps[:ln, :], EXP)
    s = pool.tile([128, 1], F32, tag="sm_s")
    nc.vector.reduce_sum(s[:ln, :], att[:ln, :], axis=AX)
    rs = pool.tile([128, 1], F32, tag="sm_rs")
    nc.vector.reciprocal(rs[:ln, :], s[:ln, :])
    nc.vector.tensor_scalar(att[:ln, :], att[:ln, :], rs[:ln, :], None,
                            op0=ALU.mult)


def _build(nsteps):
    import concourse.bass as bass
    import concourse.tile as tile
    from concourse import mybir
    from contextlib import ExitStack

    F32 = mybir.dt.float32
    F32R = mybir.dt.float32r
    I32 = mybir.dt.int32
    SIG = mybir.ActivationFunctionType.Sigmoid
    TANH = mybir.ActivationFunctionType.Tanh
    ALU = mybir.AluOpType

    nc = bass.Bass()
    dt_ = lambda name, shp, dt=F32: nc.dram_tensor(name, shp, dt, kind="ExternalInput")
    enc = dt_("enc", [EXPC, T, ENC_OUT])
    toks = dt_("toks", [1024, 1], I32)
    maskadd = dt_("maskadd", [1, EXPC * T])
    E_h = dt_("E_h", [VOCAB, EMB])
    ET_h = dt_("ET_h", [128, 2, VOCAB])
    Wk_h = dt_("Wk_h", [128, 8, KEY])
    Wv_h = dt_("Wv_h", [128, 8, KEY])
    Wq_h = dt_("Wq_h", [128, 2, KEY])
    Wih1T_h = dt_("Wih1T_h", [128, 4, 4 * DEC_H])
    Whh1T_h = dt_("Whh1T_h", [128, 4, 4 * DEC_H])
    b1_h = dt_("b1_h", [1, 4 * DEC_H])
    Wih2T_h = dt_("Wih2T_h", [128, 4, 4 * KEY])
    Whh2T_h = dt_("Whh2T_h", [128, 2, 4 * KEY])
    b2_h = dt_("b2_h", [1, 4 * KEY])
    Wfc_h = dt_("Wfc_h", [128, 4, EMB])
    bq_h = dt_("bq_h", [128, 2, 1])
    bfc_h = dt_("bfc_h", [128, 2, 1])
    bchar_h = dt_("bchar_h", [1, VOCAB])
    ident_h = dt_("ident_h", [128, 128])
    identb_h = dt_("identb_h", [128, EXPC])

    preds = nc.dram_tensor("preds", [EXPC, L, VOCAB], F32, kind="ExternalOutput")
    attns = nc.dram_tensor("attns", [L, T], F32, kind="ExternalOutput")

    with tile.TileContext(nc) as tc, ExitStack() as ctx:
        res = ctx.enter_context(tc.tile_pool(name="res", bufs=1))
        dram = ctx.enter_context(tc.tile_pool(name="dram", bufs=1, space="DRAM"))

        ident = res.tile([128, 128], F32)
        nc.gpsimd.dma_start(ident[:], ident_h[:])
        identR = res.tile([EXPC, EXPC], F32R)
        nc.gpsimd.dma_start(identR[:], ident_h[:EXPC, :EXPC])
        identB = res.tile([128, EXPC], F32)
        nc.gpsimd.dma_start(identB[:], identb_h[:])
        KT = res.tile([128, 2, EXPC, T], F32R)     # k^T: [emb_kt, ex, T]
        V = res.tile([128, 4, EXPC, KEY], F32R)    # v: [T-tile, ex, emb]
        H2T = res.tile([128, 2, EXPC, LP], F32R)   # h2^T for all steps
        nc.vector.memset(H2T[:].bitcast(F32), 0.0)
        mask_t = res.tile([1, EXPC * T], F32R)
        nc.gpsimd.dma_start(mask_t[:], maskadd[:])
        onesR = res.tile([1, 128], F32R)
        nc.vector.memset(onesR[:].bitcast(F32), 1.0)
        Wq = res.tile([128, 2, KEY], F32R)
        nc.gpsimd.dma_start(Wq[:], Wq_h[:])
        bq = res.tile([128, 2, 1], F32)
        nc.gpsimd.dma_start(bq[:], bq_h[:])

        PRE1 = dram.tile([EXPC, L, 4 * DEC_H], F32)

        # =========== PHASE A ===========
        with tc.tile_pool(name="pa", bufs=1) as pa, \
             tc.tile_pool(name="pa1", bufs=1) as pa1, \
             tc.tile_pool(name="pa2", bufs=2) as pa2, \
             tc.tile_pool(name="paps", bufs=4, space="PSUM") as paps:
            Wk = pa.tile([128, 8, KEY], F32R)
            nc.gpsimd.dma_start(Wk[:], Wk_h[:])
            Wv = pa.tile([128, 8, KEY], F32R)
            nc.gpsimd.dma_start(Wv[:], Wv_h[:])

            for ex in range(EXPC):
                encF = pa1.tile([128, 8, T], F32, tag="encF")
                for a in range(8):
                    nc.sync.dma_start(
                        encF[:, a, :],
                        enc[ex, :, a * 128:(a + 1) * 128].rearrange("t p -> p t"))
                encT = pa1.tile([128, 8, T], F32R, tag="encT")
                nc.vector.tensor_copy(encT[:], encF[:])
                for mt in range(2):
                    ps = paps.tile([128, T], F32, space="PSUM", tag="ps")
                    for kt in range(8):
                        nc.tensor.matmul(
                            ps[:], (Wk[:, kt, mt * 128:(mt + 1) * 128]),
                            (encT[:, kt, :]),
                            start=(kt == 0), stop=(kt == 7))
                    nc.vector.tensor_copy(KT[:, mt, ex, :], ps[:])
                for tt in range(4):
                    ps = paps.tile([128, KEY], F32, space="PSUM", tag="ps")
                    for kt in range(8):
                        nc.tensor.matmul(
                            ps[:], (encT[:, kt, tt * 128:(tt + 1) * 128]),
                            (Wv[:, kt, :]),
                            start=(kt == 0), stop=(kt == 7))
                    nc.vector.tensor_copy(V[:, tt, ex, :], ps[:])

            # char embedding gather + transpose into INPT tiles 0-1
            INPT = pa.tile([128, 4, EXPC, LP], F32R)
            nc.vector.memset(INPT[:].bitcast(F32), 0.0)
            tok_t = pa.tile([128, 8, 1], I32)
            nc.gpsimd.dma_start(
                tok_t[:], toks.rearrange("(a p) b -> p a b", p=128))
            ce = pa.tile([128, 8, EMB], F32)
            for g in range(8):
                nc.gpsimd.indirect_dma_start(
                    out=ce[:, g, :], out_offset=None, in_=E_h[:],
                    in_offset=bass.IndirectOffsetOnAxis(ap=tok_t[:, g, :], axis=0))
            for ex in range(EXPC):
                for lt, (lo, ln) in enumerate(LB):
                    col = ex * 2 + lt
                    for et in range(2):
                        ps = paps.tile([128, 128], F32, space="PSUM", tag="ps")
                        nc.tensor.transpose(
                            ps[:, :ln], ce[:ln, col, et * 128:(et + 1) * 128],
                            ident[:ln, :ln])
                        nc.vector.tensor_copy(INPT[:, et, ex, lo:lo + ln],
                                              ps[:, :ln])

            # Q1^T (+bq)
            Q1T = pa.tile([128, 2, EXPC, LP], F32R)
            for ex in range(EXPC):
                for mt in range(2):
                    ps = paps.tile([128, LP], F32, space="PSUM", tag="ps")
                    for kt in range(2):
                        nc.tensor.matmul(
                            ps[:], (Wq[:, kt, mt * 128:(mt + 1) * 128]),
                            (INPT[:, kt, ex, :]),
                            start=(kt == 0), stop=(kt == 1))
                    nc.vector.tensor_scalar(
                        Q1T[:, mt, ex, :], ps[:], bq[:, mt, :], None, op0=ALU.add)

            # attention 1 (+ ctx1^T per example)
            for ex in range(EXPC):
                AT1T = pa2.tile([128, 4, LP], F32R, tag="at1")
                nc.vector.memset(AT1T[:].bitcast(F32), 0.0)
                for (lo, ln) in LB:
                    eps = paps.tile([128, T], F32, space="PSUM", tag="ps")
                    for kt in range(2):
                        nc.tensor.matmul(
                            eps[:ln, :], (Q1T[:, kt, ex, lo:lo + ln]),
                            (KT[:, kt, ex, :]),
                            start=(kt == 0), stop=False)
                    nc.tensor.matmul(
                        eps[:ln, :], (onesR[:, :ln]),
                        (mask_t[:, ex * T:(ex + 1) * T]),
                        start=False, stop=True)
                    att = pa2.tile([128, T], F32, tag="att")
                    _softmax(nc, pa2, eps, att, ln)
                    for tt in range(4):
                        tps = paps.tile([128, 128], F32, space="PSUM", tag="ps")
                        nc.tensor.transpose(
                            tps[:, :ln], att[:ln, tt * 128:(tt + 1) * 128],
                            ident[:ln, :ln])
                        nc.vector.tensor_copy(
                            AT1T[:, tt, lo:lo + ln], tps[:, :ln])
                # ctx1^T -> INPT tiles 2-3
                for mt in range(2):
                    ps = paps.tile([128, LP], F32, space="PSUM", tag="ps")
                    for kt in range(4):
                        nc.tensor.matmul(
                            ps[:], (V[:, kt, ex, mt * 128:(mt + 1) * 128]),
                            (AT1T[:, kt, :]),
                            start=(kt == 0), stop=(kt == 3))
                    nc.vector.tensor_copy(INPT[:, 2 + mt, ex, :], ps[:])

            # LSTM1 pre-gates -> PRE1 (HBM)
            Wih1T = pa.tile([128, 4, 4 * DEC_H], F32R)
            nc.gpsimd.dma_start(Wih1T[:], Wih1T_h[:])
            b1 = pa.tile([1, 4 * DEC_H], F32R)
            nc.gpsimd.dma_start(b1[:], b1_h[:])
            for ex in range(EXPC):
                for (lo, ln) in LB:
                    for ch in range(4):
                        ps = paps.tile([128, 512], F32, space="PSUM", tag="ps")
                        for kt in range(4):
                            nc.tensor.matmul(
                                ps[:ln, :], (INPT[:, kt, ex, lo:lo + ln]),
                                (Wih1T[:, kt, ch * 512:(ch + 1) * 512]),
                                start=(kt == 0), stop=False)
                        nc.tensor.matmul(
                            ps[:ln, :], (onesR[:, :ln]),
                            (b1[:, ch * 512:(ch + 1) * 512]),
                            start=False, stop=True)
                        st = pa2.tile([128, 512], F32, tag="prest")
                        nc.vector.tensor_copy(st[:ln, :], ps[:ln, :])
                        nc.sync.dma_start(
                            PRE1[ex, lo:lo + ln, ch * 512:(ch + 1) * 512],
                            st[:ln, :])

        # =========== PHASE B: recurrence ===========
        with tc.tile_pool(name="pb", bufs=1) as pb, \
             tc.tile_pool(name="pb3", bufs=2) as pb3, \
             tc.tile_pool(name="pbps", bufs=1, space="PSUM") as pbps, \
             tc.tile_pool(name="pbpt", bufs=2, space="PSUM") as pbpt:
            Whh1T = pb.tile([128, 4, 4 * DEC_H], F32R)
            nc.gpsimd.dma_start(Whh1T[:], Whh1T_h[:])
            Wih2T = pb.tile([128, 4, 4 * KEY], F32R)
            nc.gpsimd.dma_start(Wih2T[:], Wih2T_h[:])
            Whh2T = pb.tile([128, 2, 4 * KEY], F32R)
            nc.gpsimd.dma_start(Whh2T[:], Whh2T_h[:])
            b2 = pb.tile([1, 4 * KEY], F32R)
            nc.gpsimd.dma_start(b2[:], b2_h[:])
            ones1 = pb.tile([1, 4], F32R)
            nc.vector.memset(ones1[:].bitcast(F32), 1.0)

            h1T = pb.tile([128, 4, EXPC], F32R)
            nc.vector.memset(h1T[:].bitcast(F32), 0.0)
            c1 = pb.tile([EXPC, 512], F32)
            nc.vector.memset(c1[:], 0.0)
            c2 = pb.tile([EXPC, 256], F32)
            nc.vector.memset(c2[:], 0.0)

            for l in range(nsteps):
                pre_l = pb3.tile([EXPC, 4 * DEC_H], F32R, tag="prel")
                nc.gpsimd.dma_start(pre_l[:], PRE1[:, l, :])

                # LSTM1 gates: [4, 2048] psum (layout [i,f,o,g])
                g1 = pbps.tile([EXPC, 2048], F32, space="PSUM", tag="g1")
                for ch in range(4):
                    o = g1[:, ch * 512:(ch + 1) * 512]
                    nc.tensor.matmul(
                        o, identR[:], (pre_l[:, ch * 512:(ch + 1) * 512]),
                        start=True, stop=False)
                    for kt in range(4):
                        nc.tensor.matmul(
                            o, (h1T[:, kt, :]),
                            (Whh1T[:, kt, ch * 512:(ch + 1) * 512]),
                            start=False, stop=(kt == 3))
                s1 = pb3.tile([EXPC, 2048], F32, tag="s1")
                nc.scalar.activation(s1[:, 0:1024], g1[:, 0:1024], SIG)
                m1 = pb3.tile([EXPC, 512], F32, tag="m1")
                nc.vector.tensor_mul(m1[:], s1[:, 512:1024], c1[:])
                nc.scalar.activation(s1[:, 1024:2048], g1[:, 1024:2048], SIG)
                t1 = pb3.tile([EXPC, 512], F32, tag="t1")
                nc.vector.tensor_scalar(t1[:], s1[:, 1536:2048], 2.0, -1.0,
                                        op0=ALU.mult, op1=ALU.add)
                nc.vector.tensor_mul(t1[:], t1[:], s1[:, 0:512])
                nc.vector.tensor_add(c1[:], m1[:], t1[:])
                th1 = pb3.tile([EXPC, 512], F32, tag="th1")
                nc.scalar.activation(th1[:], c1[:], TANH)
                h1q = pb3.tile([EXPC, 512], F32, tag="h1q")
                nc.vector.tensor_mul(h1q[:], th1[:], s1[:, 1024:1536])
                for kt in range(4):
                    tps = pbpt.tile([128, EXPC], F32, space="PSUM", tag="tp")
                    nc.tensor.transpose(
                        tps[:], h1q[:, kt * 128:(kt + 1) * 128],
                        ident[:EXPC, :EXPC])
                    nc.vector.tensor_copy(h1T[:, kt, :], tps[:])

                # LSTM2 gates: [4, 1024] psum
                g2 = pbps.tile([EXPC, 1024], F32, space="PSUM", tag="g2")
                h2prev = H2T[:, :, :, l - 1] if l > 0 else H2T[:, :, :, LP - 1]
                for ch in range(2):
                    o = g2[:, ch * 512:(ch + 1) * 512]
                    nc.tensor.matmul(
                        o, (ones1[:]), (b2[:, ch * 512:(ch + 1) * 512]),
                        start=True, stop=False)
                    for kt in range(4):
                        nc.tensor.matmul(
                            o, (h1T[:, kt, :]),
                            (Wih2T[:, kt, ch * 512:(ch + 1) * 512]),
                            start=False, stop=False)
                    for kt in range(2):
                        nc.tensor.matmul(
                            o, (h2prev[:, kt, :]),
                            (Whh2T[:, kt, ch * 512:(ch + 1) * 512]),
                            start=False, stop=(kt == 1))
                s2 = pb3.tile([EXPC, 1024], F32, tag="s2")
                nc.scalar.activation(s2[:, 0:512], g2[:, 0:512], SIG)
                m2 = pb3.tile([EXPC, 256], F32, tag="m2")
                nc.vector.tensor_mul(m2[:], s2[:, 256:512], c2[:])
                nc.scalar.activation(s2[:, 512:1024], g2[:, 512:1024], SIG)
                t2 = pb3.tile([EXPC, 256], F32, tag="t2")
                nc.vector.tensor_scalar(t2[:], s2[:, 768:1024], 2.0, -1.0,
                                        op0=ALU.mult, op1=ALU.add)
                nc.vector.tensor_mul(t2[:], t2[:], s2[:, 0:256])
                nc.vector.tensor_add(c2[:], m2[:], t2[:])
                th2 = pb3.tile([EXPC, 256], F32, tag="th2")
                nc.scalar.activation(th2[:], c2[:], TANH)
                h2q = pb3.tile([EXPC, 256], F32, tag="h2q")
                nc.vector.tensor_mul(h2q[:], th2[:], s2[:, 512:768])
                for half in range(2):
                    tps = pbpt.tile([128, EXPC], F32, space="PSUM", tag="tp")
                    nc.tensor.transpose(
                        tps[:], h2q[:, half * 128:(half + 1) * 128],
                        ident[:EXPC, :EXPC])
                    nc.vector.tensor_copy(H2T[:, half, :, l], tps[:])

        # =========== PHASE C ===========
        with tc.tile_pool(name="pc", bufs=1) as pcp, \
             tc.tile_pool(name="pc2", bufs=2) as pc2, \
             tc.tile_pool(name="pc4", bufs=4) as pc4, \
             tc.tile_pool(name="pcps", bufs=4, space="PSUM") as pcps:
            Q2T = pcp.tile([128, 2, EXPC, LP], F32R)
            for ex in range(EXPC):
                for mt in range(2):
                    ps = pcps.tile([128, LP], F32, space="PSUM", tag="ps")
                    for kt in range(2):
                        nc.tensor.matmul(
                            ps[:], (Wq[:, kt, mt * 128:(mt + 1) * 128]),
                            (H2T[:, kt, ex, :]),
                            start=(kt == 0), stop=(kt == 1))
                    nc.vector.tensor_scalar(
                        Q2T[:, mt, ex, :], ps[:], bq[:, mt, :], None, op0=ALU.add)

            CT2T = pcp.tile([128, 2, EXPC, LP], F32R)
            for ex in range(EXPC):
                AT2T = pc2.tile([128, 4, LP], F32R, tag="at2")
                nc.vector.memset(AT2T[:].bitcast(F32), 0.0)
                for (lo, ln) in LB:
                    eps = pcps.tile([128, T], F32, space="PSUM", tag="ps")
                    for kt in range(2):
                        nc.tensor.matmul(
                            eps[:ln, :], (Q2T[:, kt, ex, lo:lo + ln]),
                            (KT[:, kt, ex, :]),
                            start=(kt == 0), stop=False)
                    nc.tensor.matmul(
                        eps[:ln, :], (onesR[:, :ln]),
                        (mask_t[:, ex * T:(ex + 1) * T]),
                        start=False, stop=True)
                    att = pc2.tile([128, T], F32, tag="att2")
                    _softmax(nc, pc2, eps, att, ln)
                    if ex == 0:
                        nc.sync.dma_start(attns[lo:lo + ln, :], att[:ln, :])
                    for tt in range(4):
                        tps = pcps.tile([128, 128], F32, space="PSUM", tag="ps")
                        nc.tensor.transpose(
                            tps[:, :ln], att[:ln, tt * 128:(tt + 1) * 128],
                            ident[:ln, :ln])
                        nc.vector.tensor_copy(
                            AT2T[:, tt, lo:lo + ln], tps[:, :ln])
                for mt in range(2):
                    ps = pcps.tile([128, LP], F32, space="PSUM", tag="ps")
                    for kt in range(4):
                        nc.tensor.matmul(
                            ps[:], (V[:, kt, ex, mt * 128:(mt + 1) * 128]),
                            (AT2T[:, kt, :]),
                            start=(kt == 0), stop=(kt == 3))
                    nc.vector.tensor_copy(CT2T[:, mt, ex, :], ps[:])

            # fco^T = clip(Wfc.T @ [h2; ctx2] + bfc, -1, 1)
            Wfc = pcp.tile([128, 4, EMB], F32R)
            nc.gpsimd.dma_start(Wfc[:], Wfc_h[:])
            bfc = pcp.tile([128, 2, 1], F32)
            nc.gpsimd.dma_start(bfc[:], bfc_h[:])
            FCOT = pcp.tile([128, 2, EXPC, LP], F32R)
            for ex in range(EXPC):
                for mt in range(2):
                    ps = pcps.tile([128, LP], F32, space="PSUM", tag="ps")
                    for kt in range(2):
                        nc.tensor.matmul(
                            ps[:], (Wfc[:, kt, mt * 128:(mt + 1) * 128]),
                            (H2T[:, kt, ex, :]),
                            start=(kt == 0), stop=False)
                    for kt in range(2):
                        nc.tensor.matmul(
                            ps[:], (Wfc[:, 2 + kt, mt * 128:(mt + 1) * 128]),
                            (CT2T[:, kt, ex, :]),
                            start=False, stop=(kt == 1))
                    ftmp = pc2.tile([128, LP], F32, tag="ftmp")
                    nc.vector.tensor_scalar(ftmp[:], ps[:], bfc[:, mt, :], 1.0,
                                            op0=ALU.add, op1=ALU.min)
                    nc.vector.tensor_scalar_max(FCOT[:, mt, ex, :], ftmp[:], -1.0)

            # logits: vc-outer, ET/bchar chunks streamed from HBM
            onesL = pcp.tile([1, 128], F32R)
            nc.vector.memset(onesL[:].bitcast(F32), 1.0)
            for vc in range(20):
                vn = 512 if vc < 19 else VOCAB - 19 * 512
                ET = pc2.tile([128, 2, 512], F32R, tag="etc")
                nc.gpsimd.dma_start(ET[:, :, :vn],
                                    ET_h[:, :, vc * 512:vc * 512 + vn])
                bchar = pc2.tile([1, 512], F32R, tag="bch")
                nc.gpsimd.dma_start(bchar[:, :vn],
                                    bchar_h[:, vc * 512:vc * 512 + vn])
                for ex in range(EXPC):
                    for (lo, ln) in LB:
                        ps = pcps.tile([128, 512], F32, space="PSUM", tag="ps")
                        nc.tensor.matmul(
                            ps[:ln, :vn], (onesL[:, :ln]),
                            (bchar[:, :vn]),
                            start=True, stop=False)
                        for kt in range(2):
                            nc.tensor.matmul(
                                ps[:ln, :vn], (FCOT[:, kt, ex, lo:lo + ln]),
                                (ET[:, kt, :vn]),
                                start=False, stop=(kt == 1))
                        st = pc4.tile([128, 512], F32, tag="lgst")
                        if ex % 2 == 0:
                            nc.scalar.copy(st[:ln, :vn], ps[:ln, :vn])
                        else:
                            nc.vector.tensor_copy(st[:ln, :vn], ps[:ln, :vn])
                        nc.sync.dma_start(
                            preds[ex, lo:lo + ln, vc * 512:vc * 512 + vn],
                            st[:ln, :vn])

    orig = nc.to_json_bytes
    nc.to_json_bytes = lambda: _fix_sync_waits(orig())
    return nc


def _prep(inputs):
    f = np.float32
    enc = np.asarray(inputs["encoder_out"], f)
    text = np.asarray(inputs["text"]).astype(np.int32)
    lens = np.asarray(inputs["lens"]).astype(np.int64)
    E = np.asarray(inputs["E"], f)
    perm1 = _gate_perm(DEC_H, 1)
    perm2 = _gate_perm(KEY, 1)

    def t_kt(w):  # [in, out] -> [128, in//128, out]
        return np.ascontiguousarray(
            w.reshape(-1, 128, w.shape[1]).transpose(1, 0, 2))

    W1 = np.asarray(inputs["W_ih1"], f)[perm1]
    Wh1 = np.asarray(inputs["W_hh1"], f)[perm1]
    b1 = (np.asarray(inputs["b_ih1"], f) + np.asarray(inputs["b_hh1"], f))[perm1]
    W2 = np.asarray(inputs["W_ih2"], f)[perm2]
    Wh2 = np.asarray(inputs["W_hh2"], f)[perm2]
    b2 = (np.asarray(inputs["b_ih2"], f) + np.asarray(inputs["b_hh2"], f))[perm2]
    # pre-scale g rows by 2 (sigmoid-only trick); layout [i,f,o,g]
    W1[1536:2048] *= 2.0; Wh1[1536:2048] *= 2.0; b1[1536:2048] *= 2.0
    W2[768:1024] *= 2.0; Wh2[768:1024] *= 2.0; b2[768:1024] *= 2.0

    com = {
        "E_h": E,
        "ET_h": np.ascontiguousarray(E.reshape(VOCAB, 2, 128).transpose(2, 1, 0)),
        "Wk_h": t_kt(np.asarray(inputs["Wk"], f)),
        "Wv_h": t_kt(np.asarray(inputs["Wv"], f)),
        "Wq_h": t_kt(np.asarray(inputs["Wq"], f)),
        "Wih1T_h": t_kt(np.ascontiguousarray(W1.T)),
        "Whh1T_h": t_kt(np.ascontiguousarray(Wh1.T)),
        "b1_h": b1.reshape(1, -1),
        "Wih2T_h": t_kt(np.ascontiguousarray(W2.T)),
        "Whh2T_h": t_kt(np.ascontiguousarray(Wh2.T)),
        "b2_h": b2.reshape(1, -1),
        "Wfc_h": t_kt(np.asarray(inputs["Wfc"], f)),
        "bq_h": np.ascontiguousarray(
            np.asarray(inputs["bq"], f).reshape(2, 128).T).reshape(128, 2, 1),
        "bfc_h": np.ascontiguousarray(
            np.asarray(inputs["bfc"], f).reshape(2, 128).T).reshape(128, 2, 1),
        "bchar_h": np.asarray(inputs["b_char"], f).reshape(1, -1),
        "ident_h": np.eye(128, dtype=f),
        "identb_h": np.tile(np.eye(EXPC, dtype=f), (32, 1)),
    }
    tok = np.zeros((N, L), np.int32)
    tok[:, 1:] = text[:, :L - 1]
    mrow = np.arange(T)[None, :] >= lens[:, None]
    madd = np.where(mrow, np.float32(NEG), np.float32(0.0)).astype(f)

    in_maps = []
    for c in range(NCORES):
        sl = slice(c * EXPC, (c + 1) * EXPC)
        tk = np.zeros((1024, 1), np.int32)
        tksub = tok[sl]
        for ex in range(EXPC):
            for lt, (lo, ln) in enumerate(LB):
                col = ex * 2 + lt
                tk[col * 128:col * 128 + ln, 0] = tksub[ex, lo:lo + ln]
        m = dict(com)
        m["enc"] = np.ascontiguousarray(enc[sl])
        m["toks"] = tk
        m["maskadd"] = np.ascontiguousarray(madd[sl].reshape(1, -1))
        in_maps.append(m)
    return in_maps


def kernel(**inputs):
    from concourse.bass_utils import run_bass_kernel_spmd
    nc = _build(L)
    in_maps = _prep(inputs)
    res = run_bass_kernel_spmd(nc, in_maps, core_ids=list(range(NCORES)))
    preds = np.concatenate([r["preds"] for r in res.results], axis=0)
    attns = res.results[0]["attns"]
    return preds, attns
